# revision 7
# baseline (speedup 1.0000x reference)
"""TRN2 Bass kernel for nn_Attention_188978561266.

Reference computation (b=4, s=1024, d=1024, 16 heads x 64):
    qkv = x @ Wqkv ; split q,k,v
    q = q / (sqrt(mean(q^2 over ALL elements)) + eps) * scale_q   (global scalar RMS)
    k = k / (sqrt(mean(k^2 over ALL elements)) + eps) * scale_k
    attn = softmax(q @ k^T)  (no 1/sqrt(d_head), no mask)
    out = (attn @ v) @ Wo

Sharding: 8 cores = (batch b in 0..3) x (head-half in 0..1). Each core computes
qkv for its batch restricted to its 8 heads, full attention for those heads,
and a partial output projection in two passes (zparta = heads 0-3 of the
local half, zpartb = heads 4-7). Host sums the four partials per batch.
The global RMS needs a cross-core AllReduce of two scalars.

Schedule notes:
- dma_start costs ~0.6us serial issue time on the Sync engine, so inputs are
  shaped host-side to need few, large transfers (per-dc weight blocks are
  contiguous).
- Q/K projections and S logits run fp32r (exp amplifies absolute logit error;
  bf16 there costs ~2% output error). V/O projections and AV run bf16.
- S logit matmuls are 64-row pairs on row groups (0,0)/(64,0) -> concurrent.
- The first collective on a fresh NEFF execution takes ~60-80us of firmware
  boot; a dummy AllReduce at kernel start absorbs it concurrently with the
  projections, and warm-keeper matmuls bridge the PE to the real AllReduce's
  completion so the clock gate stays hot into attention.
- Attention runs per (head-pair g, q-half t) wave: 16 S matmuls -> 4 exps of
  2048 elems (whole 4-bank psum tile per call) -> 16 AV matmuls (ones column
  in V gives softmax denominators). AV psum is copied to SBUF immediately so
  the pool slot recycles without waiting for the normalize chain.
- O-projection pass 1 (head pairs g=0,1) is interleaved into the last four
  waves' PE slack; pass 2 runs at the tail into a second output tensor.
"""

import os as _os
import sys

sys.path.insert(0, "/opt/trn_rl_repo")

import numpy as np
from ml_dtypes import bfloat16

import concourse.bacc as bacc
import concourse.mybir as mybir
from concourse import library_config, tile
from concourse.bass_utils import run_bass_kernel_spmd

F32 = mybir.dt.float32
F32R = mybir.dt.float32r
BF16 = mybir.dt.bfloat16
AF = mybir.ActivationFunctionType
ALU = mybir.AluOpType
AX = mybir.AxisListType

P = 128
D = 1024
S = 1024
N_HEAD = 16
DH = 64
NHL = 8          # heads per core
DC = 8           # d contraction chunks of 128
EPS = 1e-6
COUNT = 4 * 1024 * 1024   # elements of the full q (or k) tensor
N_KEEP = 64               # warm-keeper matmuls bridging to the AllReduce
N_CORES = int(_os.environ.get("KN_CORES", "8"))
REPLICAS = [list(range(N_CORES))]

_CACHE = {}


def _rne11(x: np.ndarray) -> np.ndarray:
    """Round float32 to 11 explicit mantissa bits (matches HW float32r)."""
    u = np.ascontiguousarray(x, dtype=np.float32).view(np.uint32).astype(np.uint64)
    shift = 12
    bias = ((u >> shift) & 1) + ((1 << (shift - 1)) - 1)
    return (((u + bias) >> shift) << shift).astype(np.uint32).view(np.float32)


def _build():
    nc = bacc.Bacc("TRN2", target_bir_lowering=False, debug=False, num_devices=N_CORES)

    xt = nc.dram_tensor("xt", [P, DC, S], F32R, kind="ExternalInput")
    xb = nc.dram_tensor("xb", [P, DC, S], BF16, kind="ExternalInput")
    wqa = nc.dram_tensor("wqa", [P, DC, 4, P], F32R, kind="ExternalInput")
    wqb = nc.dram_tensor("wqb", [P, 2, DC, 2, P], F32R, kind="ExternalInput")
    wv = nc.dram_tensor("wv", [P, DC, NHL * DH], BF16, kind="ExternalInput")
    wo = nc.dram_tensor("wo", [P, 4, D], BF16, kind="ExternalInput")
    qscale = nc.dram_tensor("qscale", [P, 4], F32, kind="ExternalInput")
    zparta = nc.dram_tensor("zparta", [S, D], F32, kind="ExternalOutput")
    zpartb = nc.dram_tensor("zpartb", [S, D], F32, kind="ExternalOutput")

    with tile.TileContext(nc) as tc:
        with (
            tc.tile_pool(name="big", bufs=1) as big,
            tc.tile_pool(name="ep", bufs=3) as ep,
            tc.tile_pool(name="zp", bufs=2) as zp,
            tc.tile_pool(name="scr", bufs=2) as scrp,
            tc.tile_pool(name="ob", bufs=1) as obp,
            tc.tile_pool(name="small", bufs=2) as smallp,
            tc.tile_pool(name="stats", bufs=1) as stp,
            tc.tile_pool(name="ps", bufs=2, space="PSUM") as psp,
            tc.tile_pool(name="dram", bufs=1, space="DRAM") as dramp,
        ):
            # ---- persistent SBUF tensors ----
            xT = big.tile([P, DC, S], F32R, tag="xT")
            xB = big.tile([P, DC, S], BF16, tag="xB")
            wqsA = big.tile([P, DC, 4, P], F32R, tag="wqsA")
            wqsB = big.tile([P, 2, DC, 2, P], F32R, tag="wqsB")
            QT = big.tile([P, 4, S], F32R, tag="QT")
            KT = big.tile([P, 4, S], F32R, tag="KT")
            Vt = big.tile([P, 8, NHL, DH + 1], BF16, tag="Vt")
            zG = big.tile([P, 4, S], BF16, tag="zG")
            Wo_sb = big.tile([P, 4, D], BF16, tag="Wo")
            Wv_sb = big.tile([P, DC, NHL * DH], BF16, tag="Wv")

            qs_sb = stp.tile([P, 4], F32, tag="qs")
            sq_acc = stp.tile([P, 8], F32, tag="sqacc")
            qk2 = stp.tile([P, 2], F32, tag="qk2")
            g_sb = stp.tile([2, 1], F32, tag="gsb")
            gsum = stp.tile([1, 2], F32, tag="gsum")
            sc_a = stp.tile([1, 2], F32, tag="sca")
            sc_b = stp.tile([1, 2], F32, tag="scb")
            sc_c = stp.tile([1, 2], F32, tag="scc")
            pm = stp.tile([1, 1], F32, tag="pm")
            cinv = stp.tile([1, 1], F32, tag="cinv")
            c_bc = stp.tile([P, 1], F32, tag="cbc")
            ones_col = stp.tile([P, 1], F32, tag="ones_col")
            ones_blk = stp.tile([P, 8, NHL, 1], F32, tag="ones_blk")

            nc.gpsimd.load_library(library_config.attn)
            cc_warm_in = dramp.tile([2, 1], F32, tag="ccwi")
            cc_warm_out = dramp.tile([2, 1], F32, tag="ccwo", addr_space="Shared")
            nc.gpsimd.collective_compute(
                "AllReduce",
                ALU.add,
                replica_groups=REPLICAS,
                ins=[cc_warm_in[:]],
                outs=[cc_warm_out[:]],
            )

            # ---- input DMAs: x chunk 0 + dc-0 weights first, then streamed ----
            nc.sync.dma_start(xT[:, 0, :], xt[:, 0, :])
            nc.sync.dma_start(wqsA[:, 0, :, :], wqa[:, 0, :, :])
            for dc in range(1, DC):
                nc.sync.dma_start(xT[:, dc, :], xt[:, dc, :])
                nc.sync.dma_start(wqsA[:, dc, :, :], wqa[:, dc, :, :])
            nc.sync.dma_start(qs_sb[:], qscale[:])
            nc.vector.memset(ones_col[:], 1.0)
            nc.vector.memset(ones_blk[:], 1.0)
            nc.vector.tensor_copy(Vt[:, :, :, DH : DH + 1], ones_blk[:])
            for pair in range(2):
                nc.sync.dma_start(wqsB[:, pair, :, :, :], wqb[:, pair, :, :, :])
            for half in range(2):
                nc.sync.dma_start(
                    xB[:, 4 * half : 4 * half + 4, :], xb[:, 4 * half : 4 * half + 4, :]
                )
            nc.sync.dma_start(Wv_sb[:], wv[:])
            nc.sync.dma_start(Wo_sb[:], wo[:])

            # ---- phase A pass Q: dc-outer so matmuls start after chunk 0 ----
            tQ = [psp.tile([P, 4, 512], F32, tag="ps4", name=f"tQ{i}") for i in (0, 1)]
            for dc in range(DC):
                for ct in range(4):
                    t = tQ[ct // 2]
                    for st in range(2):
                        nc.tensor.matmul(
                            t[:, 2 * (ct % 2) + st, :],
                            lhsT=wqsA[:, dc, ct, :],
                            rhs=xT[:, dc, st * 512 : (st + 1) * 512],
                            start=(dc == 0),
                            stop=(dc == DC - 1),
                        )
            for ct in range(4):
                view = tQ[ct // 2][:, 2 * (ct % 2) : 2 * (ct % 2) + 2, :]
                scr = scrp.tile([P, 2, 512], F32, tag="scr", name=f"sq{ct}")
                nc.scalar.activation(
                    scr[:], view, AF.Square, accum_out=sq_acc[:, ct : ct + 1]
                )
                nc.vector.tensor_scalar(
                    QT[:, ct, :],
                    view.rearrange("p a b -> p (a b)"),
                    qs_sb[:, ct : ct + 1],
                    None,
                    ALU.mult,
                )

            # ---- phase A pass K: x resident, ct-pair-outer ----
            for pair in range(2):
                tK = psp.tile([P, 4, 512], F32, tag="ps4", name=f"tK{pair}")
                for dc in range(DC):
                    for cl in range(2):
                        for st in range(2):
                            nc.tensor.matmul(
                                tK[:, 2 * cl + st, :],
                                lhsT=wqsB[:, pair, dc, cl, :],
                                rhs=xT[:, dc, st * 512 : (st + 1) * 512],
                                start=(dc == 0),
                                stop=(dc == DC - 1),
                            )
                for cl in range(2):
                    ct = 4 + 2 * pair + cl
                    view = tK[:, 2 * cl : 2 * cl + 2, :]
                    scr = scrp.tile([P, 2, 512], F32, tag="scr", name=f"sk{ct}")
                    nc.scalar.activation(
                        scr[:], view, AF.Square, accum_out=sq_acc[:, ct : ct + 1]
                    )
                    nc.vector.tensor_copy(
                        KT[:, ct - 4, :], view.rearrange("p a b -> p (a b)")
                    )

            # ---- global RMS: local reduce -> AllReduce ----
            nc.vector.reduce_sum(qk2[:, 0:1], sq_acc[:, 0:4], axis=AX.X)
            nc.vector.reduce_sum(qk2[:, 1:2], sq_acc[:, 4:8], axis=AX.X)
            g_ps = psp.tile([P, 4, 512], F32, tag="ps4", name="g_ps")
            nc.tensor.matmul(
                g_ps[0:2, 0, 0:1], lhsT=qk2[:], rhs=ones_col[:], start=True, stop=True
            )
            nc.vector.tensor_copy(g_sb[:], g_ps[0:2, 0, 0:1])
            cc_in = dramp.tile([2, 1], F32, tag="ccin")
            cc_out = dramp.tile([2, 1], F32, tag="ccout", addr_space="Shared")
            nc.sync.dma_start(cc_in[:], g_sb[:])
            nc.gpsimd.collective_compute(
                "AllReduce",
                ALU.add,
                replica_groups=REPLICAS,
                ins=[cc_in[:]],
                outs=[cc_out[:]],
            )
            nc.sync.dma_start(gsum[:], cc_out[:].rearrange("a b -> b a"))

            # ---- V projection (bf16), covers part of the collective ----
            for half in range(2):
                tV = psp.tile([P, 4, 512], F32, tag="ps4", name=f"tV{half}")
                for j in range(4):
                    sm = 4 * half + j
                    for dc in range(DC):
                        nc.tensor.matmul(
                            tV[:, j, :],
                            lhsT=xB[:, dc, sm * P : (sm + 1) * P],
                            rhs=Wv_sb[:, dc, :],
                            start=(dc == 0),
                            stop=(dc == DC - 1),
                        )
                nc.vector.tensor_copy(
                    Vt[:, 4 * half : 4 * half + 4, :, 0:DH],
                    tV[:].rearrange("p a (h d) -> p a h d", h=NHL),
                )

            # ---- RMS scalar chain: sqrt(m) = exp(0.5 ln m), one Newton step.
            # (Ln+Exp live in one ACT table set; avoids loading the sqrt set.)
            nc.vector.tensor_scalar_mul(sc_a[:], gsum[:], 1.0 / COUNT)   # m
            nc.scalar.activation(sc_c[:], sc_a[:], AF.Ln)                # ln m
            nc.scalar.activation(sc_b[:], sc_c[:], AF.Exp, scale=0.5)    # r0
            nc.vector.reciprocal(sc_c[:], sc_b[:])                       # 1/r0
            nc.vector.tensor_mul(sc_c[:], sc_a[:], sc_c[:])              # m/r0
            nc.vector.tensor_add(sc_b[:], sc_b[:], sc_c[:])              # r0 + m/r0
            nc.vector.tensor_scalar(sc_b[:], sc_b[:], 0.5, EPS, ALU.mult, ALU.add)
            nc.vector.tensor_mul(pm[:], sc_b[:, 0:1], sc_b[:, 1:2])
            nc.vector.reciprocal(cinv[:], pm[:])
            nc.gpsimd.partition_broadcast(c_bc[:], cinv[:])

            # ---- attention: per (head-pair g, q-half t) wave ----
            def s_half(g, t, half, tS):
                tsl = slice(t * 512, (t + 1) * 512)
                for sk in range(4):
                    skt = 4 * half + sk
                    for i in range(2):
                        hp = i * DH
                        nc.tensor.matmul(
                            tS[i][:, sk, :],
                            lhsT=KT[hp : hp + DH, g, skt * P : (skt + 1) * P],
                            rhs=QT[hp : hp + DH, g, tsl],
                            start=True,
                            stop=True,
                        )

            def o_chunk(sm_pair, gg0, out_dram):
                tO = psp.tile([P, 4, 512], F32, tag="ps4",
                              name=f"tO_{gg0}_{sm_pair}")
                for j in range(2):
                    sm = 2 * sm_pair + j
                    for nt in range(2):
                        for gg in (gg0, gg0 + 1):
                            nc.tensor.matmul(
                                tO[:, 2 * j + nt, :],
                                lhsT=zG[:, gg, sm * P : (sm + 1) * P],
                                rhs=Wo_sb[:, gg, nt * 512 : (nt + 1) * 512],
                                start=(gg == gg0),
                                stop=(gg == gg0 + 1),
                            )
                ob = obp.tile([P, 4, 512], F32, tag="ob",
                               name=f"ob_{gg0}_{sm_pair}")
                nc.vector.tensor_copy(ob[:], tO[:])
                for j in range(2):
                    sm = 2 * sm_pair + j
                    nc.sync.dma_start(
                        out_dram[sm * P : (sm + 1) * P, :],
                        ob[:, 2 * j : 2 * j + 2, :].rearrange("p a b -> p (a b)"),
                    )

            waves = [(g, t) for g in range(4) for t in range(2)]
            for w, (g, t) in enumerate(waves):
                tsl = slice(t * 512, (t + 1) * 512)
                E_pair = [
                    ep.tile([P, 8, 512], BF16, tag="E", name=f"E_{g}_{t}_{i}")
                    for i in range(2)
                ]
                tS0 = [
                    psp.tile([P, 4, 512], F32, tag="ps4", name=f"tSa_{g}_{t}_{i}")
                    for i in range(2)
                ]
                s_half(g, t, 0, tS0)
                if w == 0:
                    # Warm keepers: the exp below gates on the AllReduce
                    # result; keep the PE busy so the clock gate stays 8/8.
                    wk = psp.tile([P, 4, 512], F32, tag="ps4", name="wk")
                    for r in range(N_KEEP):
                        nc.tensor.matmul(
                            wk[:, r % 4, :],
                            lhsT=KT[:, 0, 0:P],
                            rhs=KT[:, 0, 0:512],
                            start=True,
                            stop=True,
                        )
                for i in range(2):
                    nc.scalar.activation(
                        E_pair[i][:, 0:4, :], tS0[i][:], AF.Exp, scale=c_bc[:, 0:1]
                    )
                tS1 = [
                    psp.tile([P, 4, 512], F32, tag="ps4", name=f"tSb_{g}_{t}_{i}")
                    for i in range(2)
                ]
                s_half(g, t, 1, tS1)
                for i in range(2):
                    nc.scalar.activation(
                        E_pair[i][:, 4:8, :], tS1[i][:], AF.Exp, scale=c_bc[:, 0:1]
                    )
                tAV = psp.tile([P, 4, 512], F32, tag="ps4", name=f"tAV_{g}_{t}")
                for i in range(2):
                    l = 2 * g + i
                    for skc in range(8):
                        nc.tensor.matmul(
                            tAV[0 : DH + 1, i, :],
                            lhsT=Vt[:, skc, l, :],
                            rhs=E_pair[i][:, skc, :],
                            start=(skc == 0),
                            stop=(skc == 7),
                        )
                # copy raw z+denominator to SBUF so the psum slot recycles
                # without waiting for the normalize chain (the denominator
                # copy also shifts partition 64 -> 0 for the DVE reciprocal)
                zAV = zp.tile([DH, 2, 512], F32, tag="zav", name=f"zAV_{g}_{t}")
                zden = obp.tile([1, 2, 512], F32, tag="zden", name=f"zd_{g}_{t}")
                nc.vector.tensor_copy(zAV[:], tAV[0:DH, 0:2, :])
                nc.vector.tensor_copy(zden[:], tAV[DH : DH + 1, 0:2, :])
                for i in range(2):
                    rs_r = smallp.tile([1, 512], F32, tag="rs", name=f"rr_{g}_{t}_{i}")
                    nc.vector.reciprocal_approx_fast(rs_r[:], zden[:, i, :])
                    bc_sb = smallp.tile([DH, 512], F32, tag="bcs",
                                        name=f"bc_{g}_{t}_{i}")
                    nc.gpsimd.partition_broadcast(bc_sb[:], rs_r[:])
                    if i == 0:
                        nc.vector.tensor_mul(
                            zG[0:DH, g, tsl], zAV[0:DH, 0, :], bc_sb[:]
                        )
                    else:
                        ztmp = obp.tile([DH, 512], BF16, tag="ztmp",
                                           name=f"zt_{g}_{t}")
                        nc.vector.tensor_mul(ztmp[:], zAV[0:DH, 1, :], bc_sb[:])
                        nc.sync.dma_start(zG[DH:P, g, tsl], ztmp[:])
                # O-projection pass 1 (heads pairs 0,1) in the PE slack of
                # the last four waves
                if w >= 4:
                    o_chunk(w - 4, 0, zparta)
            # ---- O-projection pass 2 (head pairs 2,3) tail ----
            for sm_pair in range(4):
                o_chunk(sm_pair, 2, zpartb)

    nc.compile()
    return nc


def _get_nc():
    if "nc" not in _CACHE:
        _CACHE["nc"] = _build()
    return _CACHE["nc"]


def _prep_core_inputs(x, Wqkv, Wo, scale_q, scale_k):
    """Host-side shard + layout prep. Returns list of 8 in_maps."""
    x = np.asarray(x, dtype=np.float32)
    Wqkv = np.asarray(Wqkv, dtype=np.float32)
    Wo = np.asarray(Wo, dtype=np.float32)
    scale_q = np.asarray(scale_q, dtype=np.float32)
    scale_k = np.asarray(scale_k, dtype=np.float32)

    # combined per-d_head scale folded into Q (applied after raw sum-sq)
    qs_vec = np.tile(scale_q * scale_k, NHL)               # [512]
    qs_dev = np.ascontiguousarray(qs_vec.reshape(4, P).T)  # [128,4]

    xt_all, xb_all = [], []
    for b in range(4):
        xTb = x[b].T                                       # [d, s]
        lay = xTb.reshape(DC, P, S).transpose(1, 0, 2)     # [128, 8, 1024]
        xt_all.append(np.ascontiguousarray(_rne11(lay)))
        xb_all.append(np.ascontiguousarray(lay.astype(bfloat16)))

    in_maps = []
    for c in range(8):
        b = c // 2
        hh = (c % 2) * NHL
        cols = slice(hh * DH, (hh + NHL) * DH)
        wq_c = _rne11(Wqkv[:, 0 * D:1 * D][:, cols])       # [1024, 512]
        wk_c = _rne11(Wqkv[:, 1 * D:2 * D][:, cols])
        wv_c = Wqkv[:, 2 * D:3 * D][:, cols]
        # Q weights: [p, dc, ct, n] so one DMA per dc covers all 4 ct blocks
        wqa_dev = np.ascontiguousarray(
            wq_c.reshape(DC, P, 4, P).transpose(1, 0, 2, 3)
        )
        # K weights: [p, pair, dc, cl, n] so one DMA per ct-pair
        wqb_dev = np.ascontiguousarray(
            wk_c.reshape(DC, P, 2, 2, P).transpose(1, 2, 0, 3, 4)
        )
        wv_dev = np.ascontiguousarray(
            wv_c.reshape(DC, P, NHL * DH).transpose(1, 0, 2).astype(bfloat16)
        )
        # Wo rows for local heads, arranged [128, 4, 1024]:
        # chunk g partition p = head (2g + p//64), row p%64
        wo_loc = Wo[(hh * DH):(hh + NHL) * DH, :]          # [512, 1024]
        wo_dev = np.empty((P, 4, D), dtype=bfloat16)
        for g in range(4):
            wo_dev[0:DH, g, :] = wo_loc[2 * g * DH:(2 * g + 1) * DH, :].astype(bfloat16)
            wo_dev[DH:P, g, :] = wo_loc[(2 * g + 1) * DH:(2 * g + 2) * DH, :].astype(bfloat16)
        in_maps.append(
            {
                "xt": xt_all[b],
                "xb": xb_all[b],
                "wqa": wqa_dev,
                "wqb": wqb_dev,
                "wv": wv_dev,
                "wo": np.ascontiguousarray(wo_dev),
                "qscale": qs_dev,
            }
        )
    return in_maps


def run(x, Wqkv, Wo, scale_q, scale_k, trace=False):
    nc = _get_nc()
    in_maps = _prep_core_inputs(x, Wqkv, Wo, scale_q, scale_k)
    res = run_bass_kernel_spmd(
        nc, in_maps[:N_CORES], core_ids=list(range(N_CORES)), trace=trace
    )
    out = np.empty((4, S, D), dtype=np.float32)
    for b in range(4):
        if N_CORES == 8:
            out[b] = (
                res.results[2 * b]["zparta"]
                + res.results[2 * b]["zpartb"]
                + res.results[2 * b + 1]["zparta"]
                + res.results[2 * b + 1]["zpartb"]
            )
    return out, res


def kernel(x, Wqkv, Wo, scale_q, scale_k):
    out, _ = run(x, Wqkv, Wo, scale_q, scale_k, trace=False)
    return out


# revision 10
# speedup vs baseline: 1.2181x; 1.2181x over previous
"""TRN2 Bass kernel for nn_Attention_188978561266.

Reference computation (b=4, s=1024, d=1024, 16 heads x 64):
    qkv = x @ Wqkv ; split q,k,v
    q = q / (sqrt(mean(q^2 over ALL elements)) + eps) * scale_q   (global scalar RMS)
    k = k / (sqrt(mean(k^2 over ALL elements)) + eps) * scale_k
    attn = softmax(q @ k^T)  (no 1/sqrt(d_head), no mask)
    out = (attn @ v) @ Wo

Sharding: 8 cores = (batch b in 0..3) x (head-half in 0..1). Each core computes
qkv for its batch restricted to its 8 heads, full attention for those heads,
and a partial output projection in two passes (zparta = heads 0-3 of the
local half, zpartb = heads 4-7). Host sums the four partials per batch.
The global RMS needs a cross-core AllReduce of two scalars.

Schedule notes:
- dma_start costs ~0.6us serial issue time on the Sync engine, so inputs are
  shaped host-side to need few, large transfers (per-dc weight blocks are
  contiguous).
- Q/K projections and S logits run fp32r (exp amplifies absolute logit error;
  bf16 there costs ~2% output error). V/O projections and AV run bf16.
- S logit matmuls are 64-row pairs on row groups (0,0)/(64,0) -> concurrent.
- The first collective on a fresh NEFF execution takes ~60-80us of firmware
  boot; a dummy AllReduce at kernel start absorbs it concurrently with the
  projections, and warm-keeper matmuls bridge the PE to the real AllReduce's
  completion so the clock gate stays hot into attention.
- Attention runs per (head-pair g, q-half t) wave: 16 S matmuls -> 4 exps of
  2048 elems (whole 4-bank psum tile per call) -> 16 AV matmuls (ones column
  in V gives softmax denominators). AV psum is copied to SBUF immediately so
  the pool slot recycles without waiting for the normalize chain.
- O-projection pass 1 (head pairs g=0,1) is interleaved into the last four
  waves' PE slack; pass 2 runs at the tail into a second output tensor.
"""

import os as _os
import sys

sys.path.insert(0, "/opt/trn_rl_repo")

import numpy as np
from ml_dtypes import bfloat16

import concourse.bacc as bacc
import concourse.mybir as mybir
from concourse import library_config, tile
from concourse.bass_utils import run_bass_kernel_spmd

F32 = mybir.dt.float32
F32R = mybir.dt.float32r
BF16 = mybir.dt.bfloat16
AF = mybir.ActivationFunctionType
ALU = mybir.AluOpType
AX = mybir.AxisListType

P = 128
D = 1024
S = 1024
N_HEAD = 16
DH = 64
NHL = 8          # heads per core
DC = 8           # d contraction chunks of 128
EPS = 1e-6
COUNT = 4 * 1024 * 1024   # elements of the full q (or k) tensor
N_KEEP = 64               # warm-keeper matmuls bridging to the AllReduce
N_CORES = int(_os.environ.get("KN_CORES", "8"))
REPLICAS = [list(range(N_CORES))]

_CACHE = {}


def _rne11(x: np.ndarray) -> np.ndarray:
    """Round float32 to 11 explicit mantissa bits (matches HW float32r)."""
    u = np.ascontiguousarray(x, dtype=np.float32).view(np.uint32).astype(np.uint64)
    shift = 12
    bias = ((u >> shift) & 1) + ((1 << (shift - 1)) - 1)
    return (((u + bias) >> shift) << shift).astype(np.uint32).view(np.float32)


def _build():
    nc = bacc.Bacc("TRN2", target_bir_lowering=False, debug=False, num_devices=N_CORES)

    xt = nc.dram_tensor("xt", [P, DC, S], F32R, kind="ExternalInput")
    wqa = nc.dram_tensor("wqa", [P, DC, 4, P], F32R, kind="ExternalInput")
    wqb = nc.dram_tensor("wqb", [P, 2, DC, 2, P], F32R, kind="ExternalInput")
    wv = nc.dram_tensor("wv", [P, DC, NHL * DH], F32R, kind="ExternalInput")
    wo = nc.dram_tensor("wo", [P, 4, D], BF16, kind="ExternalInput")
    qscale = nc.dram_tensor("qscale", [P, 4], F32, kind="ExternalInput")
    zparta = nc.dram_tensor("zparta", [S, D], F32, kind="ExternalOutput")
    zpartb = nc.dram_tensor("zpartb", [S, D], F32, kind="ExternalOutput")

    with tile.TileContext(nc) as tc:
        with (
            tc.tile_pool(name="big", bufs=1) as big,
            tc.tile_pool(name="ep", bufs=2) as ep,
            tc.tile_pool(name="zp", bufs=1) as zp,
            tc.tile_pool(name="scr", bufs=2) as scrp,
            tc.tile_pool(name="ob", bufs=1) as obp,
            tc.tile_pool(name="small", bufs=2) as smallp,
            tc.tile_pool(name="stats", bufs=1) as stp,
            tc.tile_pool(name="ps", bufs=2, space="PSUM") as psp,
            tc.tile_pool(name="dram", bufs=1, space="DRAM") as dramp,
        ):
            # ---- persistent SBUF tensors ----
            xT = big.tile([P, DC, S], F32R, tag="xT")
            wqsA = big.tile([P, DC, 4, P], F32R, tag="wqsA")
            wqsB = big.tile([P, 2, DC, 2, P], F32R, tag="wqsB")
            QT = big.tile([P, 4, S], F32R, tag="QT")
            KT = big.tile([P, 4, S], F32R, tag="KT")
            Vt = big.tile([P, 8, NHL, DH + 1], BF16, tag="Vt")
            zG = big.tile([P, 4, S], BF16, tag="zG")
            Wo_sb = big.tile([P, 4, D], BF16, tag="Wo")
            Wv_sb = big.tile([P, DC, NHL * DH], F32R, tag="Wv")

            qs_sb = stp.tile([P, 4], F32, tag="qs")
            sq_acc = stp.tile([P, 8], F32, tag="sqacc")
            qk2 = stp.tile([P, 2], F32, tag="qk2")
            g_sb = stp.tile([2, 1], F32, tag="gsb")
            gsum = stp.tile([1, 2], F32, tag="gsum")
            sc_a = stp.tile([1, 2], F32, tag="sca")
            sc_b = stp.tile([1, 2], F32, tag="scb")
            sc_c = stp.tile([1, 2], F32, tag="scc")
            pm = stp.tile([1, 1], F32, tag="pm")
            cinv = stp.tile([1, 1], F32, tag="cinv")
            c_bc = stp.tile([P, 1], F32, tag="cbc")
            ones_col = stp.tile([P, 1], F32, tag="ones_col")
            ones_blk = stp.tile([P, 8, NHL, 1], F32, tag="ones_blk")

            nc.gpsimd.load_library(library_config.attn)
            cc_warm_in = dramp.tile([2, 1], F32, tag="ccwi")
            cc_warm_out = dramp.tile([2, 1], F32, tag="ccwo", addr_space="Shared")
            nc.gpsimd.collective_compute(
                "AllReduce",
                ALU.add,
                replica_groups=REPLICAS,
                ins=[cc_warm_in[:]],
                outs=[cc_warm_out[:]],
            )

            # ---- input DMAs: x chunk 0 + dc-0 weights first, then streamed ----
            nc.sync.dma_start(xT[:, 0, :], xt[:, 0, :])
            nc.sync.dma_start(wqsA[:, 0, :, :], wqa[:, 0, :, :])
            for dc in range(1, DC):
                nc.sync.dma_start(xT[:, dc, :], xt[:, dc, :])
                nc.sync.dma_start(wqsA[:, dc, :, :], wqa[:, dc, :, :])
            nc.sync.dma_start(qs_sb[:], qscale[:])
            nc.vector.memset(ones_col[:], 1.0)
            nc.vector.memset(ones_blk[:], 1.0)
            nc.vector.tensor_copy(Vt[:, :, :, DH : DH + 1], ones_blk[:])
            for pair in range(2):
                nc.sync.dma_start(wqsB[:, pair, :, :, :], wqb[:, pair, :, :, :])
            nc.sync.dma_start(Wv_sb[:], wv[:])
            nc.sync.dma_start(Wo_sb[:], wo[:])

            # ---- phase A pass Q: dc-outer so matmuls start after chunk 0 ----
            tQ = [psp.tile([P, 4, 512], F32, tag="ps4", name=f"tQ{i}") for i in (0, 1)]
            for dc in range(DC):
                for ct in range(4):
                    t = tQ[ct // 2]
                    for st in range(2):
                        nc.tensor.matmul(
                            t[:, 2 * (ct % 2) + st, :],
                            lhsT=wqsA[:, dc, ct, :],
                            rhs=xT[:, dc, st * 512 : (st + 1) * 512],
                            start=(dc == 0),
                            stop=(dc == DC - 1),
                        )
            for ct in range(4):
                view = tQ[ct // 2][:, 2 * (ct % 2) : 2 * (ct % 2) + 2, :]
                scr = scrp.tile([P, 2, 512], F32, tag="scr", name=f"sq{ct}")
                nc.scalar.activation(
                    scr[:], view, AF.Square, accum_out=sq_acc[:, ct : ct + 1]
                )
                nc.vector.tensor_scalar(
                    QT[:, ct, :],
                    view.rearrange("p a b -> p (a b)"),
                    qs_sb[:, ct : ct + 1],
                    None,
                    ALU.mult,
                )

            # ---- phase A pass K: x resident, ct-pair-outer ----
            for pair in range(2):
                tK = psp.tile([P, 4, 512], F32, tag="ps4", name=f"tK{pair}")
                for dc in range(DC):
                    for cl in range(2):
                        for st in range(2):
                            nc.tensor.matmul(
                                tK[:, 2 * cl + st, :],
                                lhsT=wqsB[:, pair, dc, cl, :],
                                rhs=xT[:, dc, st * 512 : (st + 1) * 512],
                                start=(dc == 0),
                                stop=(dc == DC - 1),
                            )
                for cl in range(2):
                    ct = 4 + 2 * pair + cl
                    view = tK[:, 2 * cl : 2 * cl + 2, :]
                    scr = scrp.tile([P, 2, 512], F32, tag="scr", name=f"sk{ct}")
                    nc.scalar.activation(
                        scr[:], view, AF.Square, accum_out=sq_acc[:, ct : ct + 1]
                    )
                    nc.vector.tensor_copy(
                        KT[:, ct - 4, :], view.rearrange("p a b -> p (a b)")
                    )

            # ---- global RMS: local reduce -> AllReduce ----
            nc.vector.reduce_sum(qk2[:, 0:1], sq_acc[:, 0:4], axis=AX.X)
            nc.vector.reduce_sum(qk2[:, 1:2], sq_acc[:, 4:8], axis=AX.X)
            g_ps = psp.tile([P, 4, 512], F32, tag="ps4", name="g_ps")
            nc.tensor.matmul(
                g_ps[0:2, 0, 0:1], lhsT=qk2[:], rhs=ones_col[:], start=True, stop=True
            )
            nc.vector.tensor_copy(g_sb[:], g_ps[0:2, 0, 0:1])
            cc_in = dramp.tile([2, 1], F32, tag="ccin")
            cc_out = dramp.tile([2, 1], F32, tag="ccout", addr_space="Shared")
            nc.sync.dma_start(cc_in[:], g_sb[:])
            nc.gpsimd.collective_compute(
                "AllReduce",
                ALU.add,
                replica_groups=REPLICAS,
                ins=[cc_in[:]],
                outs=[cc_out[:]],
            )
            nc.sync.dma_start(gsum[:], cc_out[:].rearrange("a b -> b a"))

            # ---- V projection (bf16), covers part of the collective ----
            for half in range(2):
                tV = psp.tile([P, 4, 512], F32, tag="ps4", name=f"tV{half}")
                for j in range(4):
                    sm = 4 * half + j
                    for dc in range(DC):
                        nc.tensor.matmul(
                            tV[:, j, :],
                            lhsT=xT[:, dc, sm * P : (sm + 1) * P],
                            rhs=Wv_sb[:, dc, :],
                            start=(dc == 0),
                            stop=(dc == DC - 1),
                        )
                nc.vector.tensor_copy(
                    Vt[:, 4 * half : 4 * half + 4, :, 0:DH],
                    tV[:].rearrange("p a (h d) -> p a h d", h=NHL),
                )

            # ---- RMS scalar chain: sqrt(m) = exp(0.5 ln m), one Newton step.
            # (Ln+Exp live in one ACT table set; avoids loading the sqrt set.)
            nc.vector.tensor_scalar_mul(sc_a[:], gsum[:], 1.0 / COUNT)   # m
            nc.scalar.activation(sc_c[:], sc_a[:], AF.Ln)                # ln m
            nc.scalar.activation(sc_b[:], sc_c[:], AF.Exp, scale=0.5)    # r0
            nc.vector.reciprocal(sc_c[:], sc_b[:])                       # 1/r0
            nc.vector.tensor_mul(sc_c[:], sc_a[:], sc_c[:])              # m/r0
            nc.vector.tensor_add(sc_b[:], sc_b[:], sc_c[:])              # r0 + m/r0
            nc.vector.tensor_scalar(sc_b[:], sc_b[:], 0.5, EPS, ALU.mult, ALU.add)
            nc.vector.tensor_mul(pm[:], sc_b[:, 0:1], sc_b[:, 1:2])
            nc.vector.reciprocal(cinv[:], pm[:])
            nc.gpsimd.partition_broadcast(c_bc[:], cinv[:])

            # ---- attention: per (head-pair g, q-half t) wave ----
            def o_chunk(sm_pair, gg0, out_dram):
                tO = psp.tile([P, 4, 512], F32, tag="ps4",
                              name=f"tO_{gg0}_{sm_pair}")
                for j in range(2):
                    sm = 2 * sm_pair + j
                    for nt in range(2):
                        for gg in (gg0, gg0 + 1):
                            nc.tensor.matmul(
                                tO[:, 2 * j + nt, :],
                                lhsT=zG[:, gg, sm * P : (sm + 1) * P],
                                rhs=Wo_sb[:, gg, nt * 512 : (nt + 1) * 512],
                                start=(gg == gg0),
                                stop=(gg == gg0 + 1),
                            )
                ob = obp.tile([P, 4, 512], F32, tag="ob",
                               name=f"ob_{gg0}_{sm_pair}")
                nc.vector.tensor_copy(ob[:], tO[:])
                for j in range(2):
                    sm = 2 * sm_pair + j
                    nc.sync.dma_start(
                        out_dram[sm * P : (sm + 1) * P, :],
                        ob[:, 2 * j : 2 * j + 2, :].rearrange("p a b -> p (a b)"),
                    )

            # E layout interleaves the two heads: chunk index 2*skc + i, so
            # each 4-bank psum tile (2 skt x 2 heads) is exactly one 2048-elem
            # exp call and the 2-slot psum ring advances quarter by quarter
            # with S pairs adjacent (concurrent 64-row tiles).
            waves = [(g, t) for g in range(4) for t in range(2)]
            for w, (g, t) in enumerate(waves):
                tsl = slice(t * 512, (t + 1) * 512)
                if w == 0:
                    # Warm keepers: the exps below gate on the AllReduce
                    # result; keep the PE busy so the clock gate stays 8/8.
                    # Allocated before the first S tile so they are not
                    # queued behind the exp-gated pool slots.
                    wk = psp.tile([P, 4, 512], F32, tag="ps4", name="wk")
                    for r in range(N_KEEP):
                        nc.tensor.matmul(
                            wk[:, r % 4, :],
                            lhsT=KT[:, 0, 0:P],
                            rhs=KT[:, 0, 0:512],
                            start=True,
                            stop=True,
                        )
                E_mix = ep.tile([P, 16, 512], BF16, tag="E", name=f"E_{g}_{t}")
                for q in range(4):
                    tS = psp.tile([P, 4, 512], F32, tag="ps4", name=f"tS_{g}_{t}_{q}")
                    for s in range(2):
                        skt = 2 * q + s
                        for i in range(2):
                            hp = i * DH
                            nc.tensor.matmul(
                                tS[:, 2 * s + i, :],
                                lhsT=KT[hp : hp + DH, g, skt * P : (skt + 1) * P],
                                rhs=QT[hp : hp + DH, g, tsl],
                                start=True,
                                stop=True,
                            )
                    nc.scalar.activation(
                        E_mix[:, 4 * q : 4 * q + 4, :], tS[:], AF.Exp,
                        scale=c_bc[:, 0:1]
                    )
                tAV = psp.tile([P, 4, 512], F32, tag="ps4", name=f"tAV_{g}_{t}")
                for i in range(2):
                    l = 2 * g + i
                    for skc in range(8):
                        nc.tensor.matmul(
                            tAV[0 : DH + 1, i, :],
                            lhsT=Vt[:, skc, l, :],
                            rhs=E_mix[:, 2 * skc + i, :],
                            start=(skc == 0),
                            stop=(skc == 7),
                        )
                # copy raw z+denominator to SBUF so the psum slot recycles
                # without waiting for the normalize chain (the denominator
                # copy also shifts partition 64 -> 0 for the DVE reciprocal)
                zAV = zp.tile([DH, 2, 512], F32, tag="zav", name=f"zAV_{g}_{t}")
                zden = obp.tile([1, 2, 512], F32, tag="zden", name=f"zd_{g}_{t}")
                nc.vector.tensor_copy(zAV[:], tAV[0:DH, 0:2, :])
                nc.vector.tensor_copy(zden[:], tAV[DH : DH + 1, 0:2, :])
                for i in range(2):
                    rs_r = smallp.tile([1, 512], F32, tag="rs", name=f"rr_{g}_{t}_{i}")
                    nc.vector.reciprocal_approx_fast(rs_r[:], zden[:, i, :])
                    bc_sb = smallp.tile([DH, 512], F32, tag="bcs",
                                        name=f"bc_{g}_{t}_{i}")
                    nc.gpsimd.partition_broadcast(bc_sb[:], rs_r[:])
                    if i == 0:
                        nc.vector.tensor_mul(
                            zG[0:DH, g, tsl], zAV[0:DH, 0, :], bc_sb[:]
                        )
                    else:
                        ztmp = obp.tile([DH, 512], BF16, tag="ztmp",
                                           name=f"zt_{g}_{t}")
                        nc.vector.tensor_mul(ztmp[:], zAV[0:DH, 1, :], bc_sb[:])
                        nc.sync.dma_start(zG[DH:P, g, tsl], ztmp[:])
                # O-projection pass 1 (heads pairs 0,1) in the PE slack of
                # the last four waves
                if w >= 4:
                    o_chunk(w - 4, 0, zparta)
            # ---- O-projection pass 2 (head pairs 2,3) tail ----
            for sm_pair in range(4):
                o_chunk(sm_pair, 2, zpartb)

    nc.compile()
    return nc


def _get_nc():
    if "nc" not in _CACHE:
        _CACHE["nc"] = _build()
    return _CACHE["nc"]


def _prep_core_inputs(x, Wqkv, Wo, scale_q, scale_k):
    """Host-side shard + layout prep. Returns list of 8 in_maps."""
    x = np.asarray(x, dtype=np.float32)
    Wqkv = np.asarray(Wqkv, dtype=np.float32)
    Wo = np.asarray(Wo, dtype=np.float32)
    scale_q = np.asarray(scale_q, dtype=np.float32)
    scale_k = np.asarray(scale_k, dtype=np.float32)

    # combined per-d_head scale folded into Q (applied after raw sum-sq)
    qs_vec = np.tile(scale_q * scale_k, NHL)               # [512]
    qs_dev = np.ascontiguousarray(qs_vec.reshape(4, P).T)  # [128,4]

    xt_all = []
    for b in range(4):
        xTb = x[b].T                                       # [d, s]
        lay = xTb.reshape(DC, P, S).transpose(1, 0, 2)     # [128, 8, 1024]
        xt_all.append(np.ascontiguousarray(_rne11(lay)))

    in_maps = []
    for c in range(8):
        b = c // 2
        hh = (c % 2) * NHL
        cols = slice(hh * DH, (hh + NHL) * DH)
        wq_c = _rne11(Wqkv[:, 0 * D:1 * D][:, cols])       # [1024, 512]
        wk_c = _rne11(Wqkv[:, 1 * D:2 * D][:, cols])
        wv_c = Wqkv[:, 2 * D:3 * D][:, cols]
        # Q weights: [p, dc, ct, n] so one DMA per dc covers all 4 ct blocks
        wqa_dev = np.ascontiguousarray(
            wq_c.reshape(DC, P, 4, P).transpose(1, 0, 2, 3)
        )
        # K weights: [p, pair, dc, cl, n] so one DMA per ct-pair
        wqb_dev = np.ascontiguousarray(
            wk_c.reshape(DC, P, 2, 2, P).transpose(1, 2, 0, 3, 4)
        )
        wv_dev = np.ascontiguousarray(
            _rne11(wv_c).reshape(DC, P, NHL * DH).transpose(1, 0, 2)
        )
        # Wo rows for local heads, arranged [128, 4, 1024]:
        # chunk g partition p = head (2g + p//64), row p%64
        wo_loc = Wo[(hh * DH):(hh + NHL) * DH, :]          # [512, 1024]
        wo_dev = np.empty((P, 4, D), dtype=bfloat16)
        for g in range(4):
            wo_dev[0:DH, g, :] = wo_loc[2 * g * DH:(2 * g + 1) * DH, :].astype(bfloat16)
            wo_dev[DH:P, g, :] = wo_loc[(2 * g + 1) * DH:(2 * g + 2) * DH, :].astype(bfloat16)
        in_maps.append(
            {
                "xt": xt_all[b],
                "wqa": wqa_dev,
                "wqb": wqb_dev,
                "wv": wv_dev,
                "wo": np.ascontiguousarray(wo_dev),
                "qscale": qs_dev,
            }
        )
    return in_maps


def run(x, Wqkv, Wo, scale_q, scale_k, trace=False):
    nc = _get_nc()
    in_maps = _prep_core_inputs(x, Wqkv, Wo, scale_q, scale_k)
    res = run_bass_kernel_spmd(
        nc, in_maps[:N_CORES], core_ids=list(range(N_CORES)), trace=trace
    )
    out = np.empty((4, S, D), dtype=np.float32)
    for b in range(4):
        if N_CORES == 8:
            out[b] = (
                res.results[2 * b]["zparta"]
                + res.results[2 * b]["zpartb"]
                + res.results[2 * b + 1]["zparta"]
                + res.results[2 * b + 1]["zpartb"]
            )
    return out, res


def kernel(x, Wqkv, Wo, scale_q, scale_k):
    out, _ = run(x, Wqkv, Wo, scale_q, scale_k, trace=False)
    return out
